# revision 36
# baseline (speedup 1.0000x reference)
"""ChebNet (K=4, 2 layers) on 8 Trainium2 NeuronCores.

Strategy (node sharding, dest-major gather):
  - Nodes sharded across 8 cores (12500/core, padded to 12544 = 98 blocks of 128).
  - Edges sharded by destination (row). Per core, edges sorted by
    (dest block, source range, source), grouped into 128-edge chunks.
  - prop(t)[r] = -dis[r] * sum_{e: row_e=r} dis[col_e] * t[col_e]
    Gather dis[col]*t[col] rows from an allgathered "scaled table" in HBM via
    dma_gather (int16 idx => 4 source ranges of <=32K padded rows each);
    segment-sum via one-hot selection matmuls accumulated in PSUM
    (lhsT = S01[e,r] one-hot built on DVE with is_equal, rhs = gathered [e,ch]);
    the -dis[r] (and 2x of the Chebyshev recurrence) fused into per-partition
    ACT scale on the PSUM->SBUF epilogue copy.
  - Layer 1 uses the T_k recurrence (2 AllGathers); layer 2 is rewritten in
    Horner form over U_k = H @ W2_k (64-ch props, 3 AllGathers):
      out2 = (U0-U2) + L[(U1-3U3) + L(2U2 + L(4U3))] + b2
  - Weight contractions run in transposed space via PE transposes; the whole
    weight phase (P4) is fused into hop3's epilogue so its PE/DVE work hides
    under hop3's gather stream.
  - dma_gather desc-gen is the critical resource: 4 SWDGE queues spread it
    over all four Q7 cpu pairs (queue q runs on cpus 2q/2q+1).
"""

import math
import numpy as np
from contextlib import ExitStack

import concourse.bass as bass
import concourse.mybir as mybir
import concourse.tile as tile
from concourse.bass_utils import run_bass_kernel_spmd

F32 = mybir.dt.float32
F16 = mybir.dt.float16
I16 = mybir.dt.int16

AF = mybir.ActivationFunctionType
ALU = mybir.AluOpType


class CFG:
    ncores = 8
    b_s = 8           # chunks per batched S01 build
    pair = 2          # dest blocks per gather call
    table_f16 = True  # scaled gather tables + S01 in fp16 (accuracy/perf knob)
    gt_bufs = 2       # gather tile double buffering
    nq = 4            # SWDGE queues (desc-gen runs on a different Q7 pair per queue)
    dma_scratch = 65536  # SWDGE desc-ring carveout bytes/partition


def _np_dt(dt):
    return {F32: np.float32, F16: np.float16, I16: np.int16}[dt]


# ----------------------------------------------------------------------------
# Host preprocessing
# ----------------------------------------------------------------------------

def preprocess(x, W1, b1, W2, b2, edge_index, cfg=CFG):
    N, IN_C = x.shape
    K, _, HID_C = W1.shape
    OUT_C = W2.shape[2]
    NCORES = cfg.ncores
    assert N % NCORES == 0
    NS = N // NCORES
    B = (NS + 127) // 128
    NSp = B * 128
    NT = NCORES * NSp

    R = max(1, math.ceil(NT / 32768))
    while NT % R:
        R += 1
    RW = NT // R
    assert RW <= 32767 or NT <= 32767, (NT, R, RW)

    row = np.asarray(edge_index[0]).astype(np.int64)
    col = np.asarray(edge_index[1]).astype(np.int64)
    deg = np.bincount(row, minlength=N).astype(np.float32)
    dis = np.where(deg > 0, 1.0 / np.sqrt(np.maximum(deg, 1.0, dtype=np.float32)),
                   np.float32(0.0)).astype(np.float32)

    tdt = F16 if cfg.table_f16 else F32
    sdt = tdt  # S01 dtype must match gathered dtype for matmul
    # L2 table row must be a multiple of 256 bytes for dma_gather
    esz = mybir.dt.size(tdt)
    L2W = OUT_C if (OUT_C * esz) % 256 == 0 else (OUT_C * esz + 255) // 256 * 256 // esz

    cp_all = (col // NS) * NSp + (col % NS)

    per_core = []
    maxcnt = 1
    for c in range(NCORES):
        sel = (row >= c * NS) & (row < (c + 1) * NS)
        r_loc = (row[sel] - c * NS).astype(np.int64)
        cp = cp_all[sel]
        blk = r_loc >> 7
        rng = cp // RW
        order = np.lexsort((cp, rng, blk))
        r_loc, cp, blk, rng = r_loc[order], cp[order], blk[order], rng[order]
        grp = blk * R + rng
        cnt = np.bincount(grp, minlength=B * R)
        maxcnt = max(maxcnt, int(cnt.max()))
        per_core.append((r_loc, cp, blk, rng, grp, cnt))

    C_G = (maxcnt + 127) // 128
    C_B = R * C_G
    NCH = B * C_B
    C_Bp = (C_B + cfg.b_s - 1) // cfg.b_s * cfg.b_s

    # gather-call chunk permutation (shared layout): (pair, range, blockinpair, j)
    perm = []
    npair = (B + cfg.pair - 1) // cfg.pair
    call_widths = []  # chunks per call
    for pi in range(npair):
        blks = [b for b in range(pi * cfg.pair, min((pi + 1) * cfg.pair, B))]
        for g in range(R):
            for b in blks:
                for j in range(C_G):
                    perm.append(b * C_B + g * C_G + j)
            call_widths.append(len(blks) * C_G)
    perm = np.array(perm, dtype=np.int64)

    in_maps = []
    for c in range(NCORES):
        r_loc, cp, blk, rng, grp, cnt = per_core[c]
        starts = np.zeros(B * R, dtype=np.int64)
        starts[1:] = np.cumsum(cnt)[:-1]
        pos = np.arange(len(r_loc)) - starts[grp]
        slot = grp * (C_G * 128) + pos

        idx_arr = np.zeros(NCH * 128, dtype=np.int16)
        idx_arr[slot] = (cp - rng * RW).astype(np.int16)
        rrv = np.full(NCH * 128, 999.0, dtype=np.float32)
        rrv[slot] = (r_loc - blk * 128).astype(np.float32)

        rr_mat = rrv.reshape(NCH, 128).T.reshape(128, B, C_B)
        rr_out = np.full((128, B, C_Bp), 999.0, dtype=np.float32)
        rr_out[:, :, :C_B] = rr_mat
        rr_out = rr_out.reshape(128, B * C_Bp).astype(_np_dt(sdt))

        idx_chunks = idx_arr.reshape(NCH, 128)[perm].reshape(-1)
        w16 = idx_chunks.reshape(-1, 16).T.copy()          # [16, NCH*8]
        idx16 = np.tile(w16, (8, 1))                        # [128, NCH*8]

        dl = np.zeros(NSp, dtype=np.float32)
        dl[:NS] = dis[c * NS:(c + 1) * NS]
        d2 = dl.reshape(B, 128).T                           # [128, B]
        disv = np.stack([-d2, -2.0 * d2, -d2 * d2, d2, 4.0 * d2], axis=2)
        disv = np.ascontiguousarray(disv.reshape(128, B * 5)).astype(np.float32)

        xsh = np.zeros((NSp, IN_C), dtype=np.float32)
        xsh[:NS] = np.asarray(x[c * NS:(c + 1) * NS], dtype=np.float32)

        in_maps.append(dict(idx16=idx16, rr=rr_out, disv=disv, xsh=xsh))

    # shared (replicated) arrays
    xs_pad = np.zeros((NT, IN_C), dtype=_np_dt(tdt))
    xsc = (np.asarray(x, np.float32) * dis[:, None]).astype(_np_dt(tdt))
    for c in range(NCORES):
        xs_pad[c * NSp:c * NSp + NS] = xsc[c * NS:(c + 1) * NS]

    iota = np.tile(np.arange(128, dtype=np.float32), cfg.b_s)
    iota = np.broadcast_to(iota, (128, cfg.b_s * 128)).astype(_np_dt(sdt)).copy()
    ident = np.eye(128, dtype=np.float32)
    W1r = np.ascontiguousarray(np.asarray(W1, np.float32).transpose(1, 0, 2)
                               .reshape(IN_C, K * HID_C))
    W2r = np.ascontiguousarray(np.asarray(W2, np.float32).transpose(1, 0, 2)
                               .reshape(HID_C, K * OUT_C))
    b1c = np.asarray(b1, np.float32).reshape(HID_C, 1)
    b2c = np.asarray(b2, np.float32).reshape(OUT_C, 1)

    shared = dict(xs_pad=xs_pad, iota=iota, ident=ident, W1r=W1r, W2r=W2r,
                  b1c=b1c, b2c=b2c)
    for m in in_maps:
        m.update(shared)

    meta = dict(N=N, IN_C=IN_C, HID_C=HID_C, OUT_C=OUT_C, K=K, NS=NS, B=B,
                NSp=NSp, NT=NT, R=R, RW=RW, C_G=C_G, C_B=C_B, NCH=NCH,
                C_Bp=C_Bp, npair=npair, L2W=L2W, tdt=tdt, sdt=sdt,
                call_widths=call_widths, NCORES=NCORES)
    return in_maps, meta


# ----------------------------------------------------------------------------
# Bass kernel
# ----------------------------------------------------------------------------

def safe_barrier(tc, nc, group=3, dma_window=10):
    """All-engine barrier (multi-wait instructions are legalized later by
    split_multi_waits, so the plain strict barrier is safe)."""
    tc.strict_bb_all_engine_barrier()


def split_multi_waits(nc):
    """Walrus/NEFF on this toolchain accepts at most ONE sync wait per
    instruction. Hoist extra waits onto same-engine NoOps inserted right
    before the instruction (engines execute their stream in order)."""
    n = 0
    for func in nc.m.functions:
        for block in func.blocks:
            newlist = []
            for inst in block.instructions:
                si = getattr(inst, "sync_info", None)
                ow = (si.on_wait or []) if si else []
                if len(ow) > 1:
                    for w in ow[:-1]:
                        nop = mybir.InstNoOp(
                            name=nc.get_next_instruction_name(),
                            engine=inst.engine, ins=[], outs=[],
                            debug=inst.debug)
                        nop.sync_info = mybir.SyncInfo(on_wait=[w],
                                                       on_update=[])
                        newlist.append(nop)
                        n += 1
                    inst.sync_info = mybir.SyncInfo(on_wait=[ow[-1]],
                                                    on_update=si.on_update)
                newlist.append(inst)
            block.instructions = newlist
    return n


def build_nc(meta, cfg=CFG, stop_after=None):
    B = meta["B"]; R = meta["R"]; RW = meta["RW"]; C_G = meta["C_G"]
    C_B = meta["C_B"]; NCH = meta["NCH"]; C_Bp = meta["C_Bp"]
    NSp = meta["NSp"]; NT = meta["NT"]; NS = meta["NS"]
    IN_C = meta["IN_C"]; HID_C = meta["HID_C"]; OUT_C = meta["OUT_C"]
    K = meta["K"]; L2W = meta["L2W"]; tdt = meta["tdt"]; sdt = meta["sdt"]
    npair = meta["npair"]; NCORES = meta["NCORES"]
    B_S = cfg.b_s

    nc = bass.Bass(num_devices=NCORES, num_swdge_queues=cfg.nq,
                   dynamic_dma_scratch_size=cfg.dma_scratch)
    rg = [list(range(NCORES))]

    # ---- I/O ----
    xs_pad = nc.declare_dram_parameter("xs_pad", [NT, IN_C], tdt, isOutput=False)
    xsh = nc.declare_dram_parameter("xsh", [NSp, IN_C], F32, isOutput=False)
    idx16 = nc.declare_dram_parameter("idx16", [128, 8 * NCH], I16, isOutput=False)
    rr = nc.declare_dram_parameter("rr", [128, B * C_Bp], sdt, isOutput=False)
    disv = nc.declare_dram_parameter("disv", [128, B * 5], F32, isOutput=False)
    iota = nc.declare_dram_parameter("iota", [128, B_S * 128], sdt, isOutput=False)
    ident = nc.declare_dram_parameter("ident", [128, 128], F32, isOutput=False)
    W1r = nc.declare_dram_parameter("W1r", [IN_C, K * HID_C], F32, isOutput=False)
    W2r = nc.declare_dram_parameter("W2r", [HID_C, K * OUT_C], F32, isOutput=False)
    b1c = nc.declare_dram_parameter("b1c", [HID_C, 1], F32, isOutput=False)
    b2c = nc.declare_dram_parameter("b2c", [OUT_C, 1], F32, isOutput=False)
    y = nc.declare_dram_parameter("y", [NSp, OUT_C], F32, isOutput=True)

    # ---- internal DRAM ----
    t1loc = nc.dram_tensor("t1loc", [NSp, HID_C], F32)
    t2loc = nc.dram_tensor("t2loc", [NSp, HID_C], F32)
    ob2 = nc.dram_tensor("ob2", [NSp, OUT_C], F32)
    mb1 = nc.dram_tensor("mb1", [NSp, OUT_C], F32)
    mb2 = nc.dram_tensor("mb2", [NSp, OUT_C], F32)
    agin = [nc.dram_tensor(f"agin{i}", [NSp, IN_C if i < 2 else L2W], tdt)
            for i in range(5)]
    tbl = [nc.dram_tensor(f"tbl{i}", [NT, IN_C if i < 2 else L2W], tdt)
           for i in range(5)]

    with tile.TileContext(nc) as tc, ExitStack() as ctx:
        singles = ctx.enter_context(tc.tile_pool(name="singles", bufs=1))
        gpool = ctx.enter_context(tc.tile_pool(name="gpool", bufs=cfg.gt_bufs))
        spool = ctx.enter_context(tc.tile_pool(name="spool", bufs=3))
        epool = ctx.enter_context(tc.tile_pool(name="epool", bufs=4))
        bpool = ctx.enter_context(tc.tile_pool(name="bpool", bufs=3))
        opool = ctx.enter_context(tc.tile_pool(name="opool", bufs=4))
        pp = ctx.enter_context(tc.tile_pool(name="pp", bufs=2, space="PSUM"))
        pt = ctx.enter_context(tc.tile_pool(name="pt", bufs=1, space="PSUM"))
        pu = ctx.enter_context(tc.tile_pool(name="pu", bufs=2, space="PSUM"))

        # ---- residents ----
        sb_idx = singles.tile([128, 8 * NCH], I16)
        nc.sync.dma_start(out=sb_idx[:], in_=idx16[:, :])
        sb_rr = singles.tile([128, B * C_Bp], sdt)
        nc.sync.dma_start(out=sb_rr[:], in_=rr[:, :])
        safe_barrier(tc, nc)
        sb_disv = singles.tile([128, B * 5], F32)
        nc.sync.dma_start(out=sb_disv[:], in_=disv[:, :])
        sb_iota = singles.tile([128, B_S * 128], sdt)
        nc.sync.dma_start(out=sb_iota[:], in_=iota[:, :])
        sb_ident = singles.tile([128, 128], F32)
        nc.sync.dma_start(out=sb_ident[:], in_=ident[:, :])
        safe_barrier(tc, nc)
        sb_W1 = singles.tile([IN_C, K * HID_C], F32)
        nc.sync.dma_start(out=sb_W1[:], in_=W1r[:, :])
        sb_W2 = singles.tile([HID_C, K * OUT_C], F32)
        nc.sync.dma_start(out=sb_W2[:], in_=W2r[:, :])
        safe_barrier(tc, nc)
        sb_b1 = singles.tile([HID_C, 1], F32)
        nc.sync.dma_start(out=sb_b1[:], in_=b1c[:, :])
        sb_b2 = singles.tile([OUT_C, 1], F32)
        nc.sync.dma_start(out=sb_b2[:], in_=b2c[:, :])
        from concourse import library_config
        nc.gpsimd.load_library(library_config.mlp)
        safe_barrier(tc, nc)

        def dv(b, v):  # per-partition dis variant for block b
            return sb_disv[:, b * 5 + v: b * 5 + v + 1]

        nidx_regs = {}

        def nidx_reg(n):
            if n not in nidx_regs:
                nidx_regs[n] = nc.gpsimd.to_reg(n)
            return nidx_regs[n]

        def ag_full(i):
            safe_barrier(tc, nc)
            nc.gpsimd.collective_compute(
                "AllGather", ALU.bypass, replica_groups=rg,
                ins=[agin[i][:, :]], outs=[tbl[i][:, :]])
            safe_barrier(tc, nc)

        # ------------------------------------------------------------------
        def prop(table, elem, epilogue, tag, mid=None):
            """One application of L-hat: gathers + one-hot matmuls per block."""
            for pi in range(npair):
                blks = list(range(pi * cfg.pair, min((pi + 1) * cfg.pair, B)))
                nb = len(blks)
                gts = []
                for g in range(R):
                    gt = gpool.tile([128, nb * C_G, elem], tdt, tag=f"gt{g}")
                    call_chunk_off = (pi * cfg.pair * C_B) + 0
                    # chunk offset of this call in permuted order:
                    # calls are laid out pair-major then range-major
                    coff = pi * cfg.pair * C_B + g * nb * C_G
                    ioff = coff * 8  # int16 cols per chunk = 128/16
                    nidx = nb * C_G * 128
                    nc.gpsimd.dma_gather(
                        gt[:],
                        table[g * RW:(g + 1) * RW, :],
                        sb_idx[:, ioff: ioff + nidx // 16],
                        nidx, nidx_reg(nidx), elem,
                        single_packet=False,
                        queue_num=(pi * R + g) % cfg.nq,
                    )
                    gts.append(gt)
                for lb, b in enumerate(blks):
                    ps = pp.tile([128, elem], F32, space="PSUM", tag="prop")
                    # batched one-hot builds for this block
                    stiles = []
                    for sb0 in range(0, C_B, B_S):
                        st = spool.tile([128, B_S * 128], sdt, tag="s01")
                        w = min(B_S, C_B - sb0)
                        rr_sl = sb_rr[:, b * C_Bp + sb0: b * C_Bp + sb0 + w]
                        rr_bc = bass.AP(tensor=rr_sl.tensor, offset=rr_sl.offset,
                                        ap=list(rr_sl.ap) + [[0, 128]])
                        nc.vector.tensor_tensor(
                            out=st[:, : w * 128].rearrange(
                                "p (b j) -> p b j", j=128),
                            in0=sb_iota[:, : w * 128].rearrange(
                                "p (b j) -> p b j", j=128),
                            in1=rr_bc, op=ALU.is_equal)
                        stiles.append(st)
                    nmm = C_B
                    for g in range(R):
                        for j in range(C_G):
                            ch = g * C_G + j
                            st = stiles[ch // B_S]
                            s_sl = st[:, (ch % B_S) * 128: (ch % B_S + 1) * 128]
                            rhs = gts[g][:, lb * C_G + j, :]
                            nc.tensor.matmul(
                                out=ps[:], lhsT=s_sl, rhs=rhs,
                                start=(ch == 0), stop=(ch == nmm - 1))
                    epilogue(b, ps)

        # ------------------------------------------------------------------
        # L1 hop 1: T1 = L x
        def ep_hop1(b, ps):
            t1t = epool.tile([128, HID_C], F32, tag="ep_a")
            nc.scalar.activation(t1t[:], ps[:], AF.Copy, scale=dv(b, 0))
            tb = epool.tile([128, IN_C], tdt, tag="ep_b")
            nc.scalar.activation(tb[:], ps[:], AF.Copy, scale=dv(b, 2))
            nc.sync.dma_start(out=t1loc[b * 128:(b + 1) * 128, :], in_=t1t[:])
            nc.sync.dma_start(out=agin[0][b * 128:(b + 1) * 128, :], in_=tb[:])

        def bail(which):
            if stop_after != which:
                return False
            z = epool.tile([128, OUT_C], F32, tag="ls_o")
            nc.vector.memset(z[:], 0.0)
            for b in range(B):
                rows = min(128, NS - b * 128)
                nc.sync.dma_start(out=y[b * 128: b * 128 + rows, :],
                                  in_=z[:rows, :])
            return True

        prop(xs_pad, IN_C, ep_hop1, "hop1")
        if bail("hop1"):
            return nc
        ag_full(0)
        if bail("ag1"):
            return nc

        # L1 hop 2: T2 = 2 L T1 - x
        def ep_hop2(b, ps):
            xb = bpool.tile([128, IN_C], F32, tag="blk_in")
            nc.sync.dma_start(out=xb[:], in_=xsh[b * 128:(b + 1) * 128, :])
            s2 = epool.tile([128, HID_C], F32, tag="ep_a")
            nc.scalar.activation(s2[:], ps[:], AF.Copy, scale=dv(b, 1))
            t2t = epool.tile([128, HID_C], F32, tag="ep_c")
            nc.vector.tensor_tensor(out=t2t[:], in0=s2[:], in1=xb[:],
                                    op=ALU.subtract)
            tb = epool.tile([128, IN_C], tdt, tag="ep_b")
            nc.scalar.activation(tb[:], t2t[:], AF.Copy, scale=dv(b, 3))
            nc.sync.dma_start(out=t2loc[b * 128:(b + 1) * 128, :], in_=t2t[:])
            nc.sync.dma_start(out=agin[1][b * 128:(b + 1) * 128, :], in_=tb[:])

        prop(tbl[0], IN_C, ep_hop2, "hop2")
        if bail("hop2"):
            return nc
        ag_full(1)

        # L1 hop 3: T3 = 2 L T2 - T1, fused with the P4 weight phase:
        # H = relu(sum_k T_k W1_k + b1); U_k = H W2_k; layer-2 bias tables.
        # Fusing hides the P4 PE/DVE work under hop3's gather stream.
        def ep_hop3(b, ps):
            t1b = bpool.tile([128, HID_C], F32, tag="blk_in")
            nc.sync.dma_start(out=t1b[:], in_=t1loc[b * 128:(b + 1) * 128, :])
            s2 = epool.tile([128, HID_C], F32, tag="ep_a")
            nc.scalar.activation(s2[:], ps[:], AF.Copy, scale=dv(b, 1))
            t3t = epool.tile([128, HID_C], F32, tag="ep_c")
            nc.vector.tensor_tensor(out=t3t[:], in0=s2[:], in1=t1b[:],
                                    op=ALU.subtract)
            xb = bpool.tile([128, HID_C], F32, tag="p4_x")
            nc.sync.dma_start(out=xb[:], in_=xsh[b * 128:(b + 1) * 128, :])
            t2b = bpool.tile([128, HID_C], F32, tag="p4_t2")
            nc.sync.dma_start(out=t2b[:], in_=t2loc[b * 128:(b + 1) * 128, :])
            srcs = [xb, t1b, t2b, t3t]
            ph = pp.tile([128, 128], F32, space="PSUM", tag="p4_h")
            for k in range(K):
                ptr = pt.tile([128, 128], F32, space="PSUM", tag="p4_tr")
                nc.tensor.transpose(out=ptr[:], in_=srcs[k][:],
                                    identity=sb_ident[:])
                tkT = bpool.tile([128, 128], F32, tag="p4_tkT")
                nc.vector.tensor_copy(out=tkT[:], in_=ptr[:])
                nc.tensor.matmul(out=ph[:],
                                 lhsT=sb_W1[:, k * HID_C:(k + 1) * HID_C],
                                 rhs=tkT[:], start=(k == 0), stop=(k == K - 1))
            hT = bpool.tile([HID_C, 128], F32, tag="p4_hT")
            nc.scalar.activation(hT[:], ph[:], AF.Relu, bias=sb_b1[:], scale=1.0)
            us = []
            for k in range(K):
                puk = pu.tile([OUT_C, 128], F32, space="PSUM", tag="p4_u")
                nc.tensor.matmul(out=puk[:],
                                 lhsT=sb_W2[:, k * OUT_C:(k + 1) * OUT_C],
                                 rhs=hT[:], start=True, stop=True)
                uk = opool.tile([OUT_C, 128], F32, tag="p4_us")
                nc.vector.tensor_copy(out=uk[:], in_=puk[:])
                us.append(uk)
            aT = opool.tile([OUT_C, 128], F32, tag="p4_c1")
            nc.vector.tensor_tensor(out=aT[:], in0=us[0][:], in1=us[2][:],
                                    op=ALU.subtract)
            nc.vector.tensor_scalar(out=aT[:], in0=aT[:], scalar1=sb_b2[:],
                                    scalar2=None, op0=ALU.add)
            u3m = opool.tile([OUT_C, 128], F32, tag="p4_c2")
            nc.vector.tensor_scalar(out=u3m[:], in0=us[3][:], scalar1=3.0,
                                    scalar2=None, op0=ALU.mult)
            bT = opool.tile([OUT_C, 128], F32, tag="p4_c3")
            nc.vector.tensor_tensor(out=bT[:], in0=us[1][:], in1=u3m[:],
                                    op=ALU.subtract)
            # transposes back to node-major + writes
            for src, dst, scale, cast in (
                (aT, ob2, 1.0, False), (bT, mb1, 1.0, False),
                (us[2], mb2, 2.0, False), (us[3], None, None, True)):
                pt2 = pt.tile([128, OUT_C], F32, space="PSUM", tag="p4_tr")
                nc.tensor.transpose(out=pt2[:], in_=src[:],
                                    identity=sb_ident[:OUT_C, :OUT_C])
                if not cast:
                    ot = epool.tile([128, OUT_C], F32, tag="p4_o")
                    nc.scalar.activation(ot[:], pt2[:], AF.Copy, scale=scale)
                    nc.sync.dma_start(out=dst[b * 128:(b + 1) * 128, :],
                                      in_=ot[:])
                else:
                    tb = epool.tile([128, L2W], tdt, tag="ep_b")
                    if L2W > OUT_C:
                        nc.vector.memset(tb[:, OUT_C:], 0.0)
                    nc.scalar.activation(tb[:, :OUT_C], pt2[:], AF.Copy,
                                         scale=dv(b, 4))
                    nc.sync.dma_start(out=agin[2][b * 128:(b + 1) * 128, :],
                                      in_=tb[:])

        prop(tbl[1], IN_C, ep_hop3, "hop3")
        if bail("hop3"):
            return nc
        ag_full(2)
        if bail("p4"):
            return nc

        # ------------------------------------------------------------------
        # L2 Horner hops: M2 = 2U2 + L M3 ; M1 = (U1-3U3) + L M2 ;
        # out2 = (U0-U2+b2) + L M1
        def ep_l2(biasbuf, agi, b, ps):
            bb = bpool.tile([128, OUT_C], F32, tag="blk_in")
            nc.sync.dma_start(out=bb[:], in_=biasbuf[b * 128:(b + 1) * 128, :])
            s2 = epool.tile([128, OUT_C], F32, tag="ep_a")
            nc.scalar.activation(s2[:], ps[:, :OUT_C], AF.Copy, scale=dv(b, 0))
            mt = epool.tile([128, OUT_C], F32, tag="ep_c")
            nc.vector.tensor_tensor(out=mt[:], in0=s2[:], in1=bb[:], op=ALU.add)
            tb = epool.tile([128, L2W], tdt, tag="ep_b")
            if L2W > OUT_C:
                nc.vector.memset(tb[:, OUT_C:], 0.0)
            nc.scalar.activation(tb[:, :OUT_C], mt[:], AF.Copy, scale=dv(b, 3))
            nc.sync.dma_start(out=agi[b * 128:(b + 1) * 128, :], in_=tb[:])

        prop(tbl[2], L2W, lambda b, ps: ep_l2(mb2, agin[3], b, ps), "hop4")
        if bail("hop4"):
            return nc
        ag_full(3)

        prop(tbl[3], L2W, lambda b, ps: ep_l2(mb1, agin[4], b, ps), "hop5")
        ag_full(4)

        # final hop + log_softmax
        def ep_final(b, ps):
            bb = bpool.tile([128, OUT_C], F32, tag="blk_in")
            nc.sync.dma_start(out=bb[:], in_=ob2[b * 128:(b + 1) * 128, :])
            s2 = epool.tile([128, OUT_C], F32, tag="ep_a")
            nc.scalar.activation(s2[:], ps[:, :OUT_C], AF.Copy, scale=dv(b, 0))
            o = epool.tile([128, OUT_C], F32, tag="ep_c")
            nc.vector.tensor_tensor(out=o[:], in0=s2[:], in1=bb[:], op=ALU.add)
            mx = opool.tile([128, 1], F32, tag="ls_m")
            nc.vector.tensor_reduce(out=mx[:], in_=o[:],
                                    axis=mybir.AxisListType.X, op=ALU.max)
            sh = epool.tile([128, OUT_C], F32, tag="ls_sh")
            nc.vector.tensor_scalar(out=sh[:], in0=o[:], scalar1=mx[:],
                                    scalar2=None, op0=ALU.subtract)
            ex = epool.tile([128, OUT_C], F32, tag="ls_ex")
            nc.scalar.activation(ex[:], sh[:], AF.Exp)
            sm = opool.tile([128, 1], F32, tag="ls_s")
            nc.vector.tensor_reduce(out=sm[:], in_=ex[:],
                                    axis=mybir.AxisListType.X, op=ALU.add)
            ls = opool.tile([128, 1], F32, tag="ls_l")
            nc.scalar.activation(ls[:], sm[:], AF.Ln)
            out = epool.tile([128, OUT_C], F32, tag="ls_o")
            nc.vector.tensor_scalar(out=out[:], in0=sh[:], scalar1=ls[:],
                                    scalar2=None, op0=ALU.subtract)
            rows = min(128, NS - b * 128)
            nc.sync.dma_start(out=y[b * 128: b * 128 + rows, :],
                              in_=out[:rows, :])

        prop(tbl[4], L2W, ep_final, "hop6")
        safe_barrier(tc, nc, dma_window=48)

    return nc


# ----------------------------------------------------------------------------
# Entry point
# ----------------------------------------------------------------------------

_CACHE = {}


def kernel(x, W1, b1, W2, b2, edge_index, cfg=CFG, want_trace=False):
    x = np.asarray(x); W1 = np.asarray(W1); b1 = np.asarray(b1)
    W2 = np.asarray(W2); b2 = np.asarray(b2)
    edge_index = np.asarray(edge_index)

    in_maps, meta = preprocess(x, W1, b1, W2, b2, edge_index, cfg)
    nc = build_nc(meta, cfg, stop_after=getattr(cfg, "stop_after", None))
    mybir.codegen_inst_isa_subclasses(nc)
    split_multi_waits(nc)
    try:
        res = run_bass_kernel_spmd(nc, in_maps, list(range(meta["NCORES"])),
                                   trace=want_trace)
    except (ImportError, ModuleNotFoundError):
        res = run_bass_kernel_spmd(nc, in_maps, list(range(meta["NCORES"])),
                                   trace=False)
    NS = meta["NS"]
    out = np.concatenate([res.results[c]["y"][:NS]
                          for c in range(meta["NCORES"])], axis=0)
    if want_trace:
        return out.astype(np.float32), res
    return out.astype(np.float32)

